# revision 58
# baseline (speedup 1.0000x reference)
"""GAT (2-layer, 3-head) forward on 8 Trainium2 NeuronCores.

Math: with LeakyReLU slope ALPHA=1.0 the edge score e_ij = s1_i + s2_j is
linear, and s1_i cancels inside the row softmax.  The masked softmax over
j therefore reduces to column weights w_j = exp(s2_j - C) restricted to
adj, giving

    h'_i = (sum_j adj_ij * w_j * h_j) / (sum_j adj_ij * w_j)

i.e. one adjacency matmul against G = [w*h | w].  Both GAT layers take
this form (the same adjacency masks both), so the whole network is two
A-matmuls plus small projections.

Sharding: rows of h' (nodes) across 8 cores; each core holds the fp8
DoubleRow-interleaved adjacency columns for its 512-row slab and
computes the slab.

Precision (tolerance 2e-2, achieved ~8e-3): the adjacency matmuls of
BOTH layers and the h2 projection run in fp8 e4m3 with DoubleRow perf
mode (each matmul contracts a 256-row pair at 2x rate).  Layer 1: G
scaled by 8; the denominator weights w by 128 as an fp8 hi+lo pair (lo
stored unscaled so hi and lo accumulate into one psum against the same
adjacency operand).  Layer 2: g2 = [w2*h2 | w2] in fp8 with the scale
folded into the exp bias (a global scale cancels in num/den).  x@W is
bf16; s2 uses a bf16 hi/lo pair of the folded u vector; its softmax max
is computed exactly on the host (negC input, pre-broadcast); layer 2
skips max subtraction (s2' stays O(1)).

Schedule: all DRAM layouts are pre-interleaved host-side so every DMA
moves multi-KB contiguous runs per partition, and the adjacency is
loaded once in the DR-interleaved fp8 form both layers consume.  The
L1 matmuls run with the adjacency slice STATIONARY, so the output lands
[i-part, features] and the softmax reciprocal is a per-partition scalar.
s2/w are computed first and the tiny w gather goes out ahead of the big
ones — it absorbs the CC-stream wind-up (~55us fixed bring-up + ~10us
first-op delay) and feeds a standalone denominator-matmul phase that
runs while the G gathers are still in flight.  Heads are processed
(1, 2, 0) with per-head AllGathers (per-head staging queues keep each
trigger's DMA-completion semaphores free of later heads' traffic).
xcat returns to f-major for h2 via six PE transposes batched through
one psum bank, deferred one step so the PE never waits on the DVE
epilogue; h2 accumulates via fp8 DoubleRow against host-pair-interleaved
Wo.  g2 is gathered once and the elu+log_softmax epilogue is inlined
per i-tile so it overlaps the remaining L2 matmuls.
"""
import sys

sys.path.insert(0, "/opt/trn_rl_repo")

import numpy as np
import ml_dtypes

import concourse.bass as bass
import concourse.bacc as bacc
import concourse.mybir as mybir
import concourse.bass_isa as bass_isa
import concourse.tile as tile
from concourse.bass_utils import run_bass_kernel_spmd

BF16 = ml_dtypes.bfloat16
F8E4 = ml_dtypes.float8_e4m3

N = 4096
F = 768
HID = 768
NH = 3
NCLS = 256
NCORES = 8
SLAB = N // NCORES          # 512 rows per core
NIT = SLAB // 128           # 4 i-tiles per core
NJT = N // 128              # 32 j-tiles
NFT = F // 128              # 6 f-tiles
NCT = NH * NFT              # 18 feature col-tiles of G
G2C = NCLS + 1              # 257 = classes + s2' column (folded u2)
C264 = 264                  # fp8 g2 row padded to 264
WCOLS = 32                  # w-column block (6 used + pad)
GA = WCOLS + HID            # gather-0 row bytes: [w cols | head0 G]
GH_TOT = NH * HID           # 2304 xcat feature rows of Wo
SG = 8.0                    # fp8 scale on G ( |G*8| << 240 )
SW = 128.0                  # fp8 scale on w (w <= 1)
S2 = 32.0                   # fp8 scale on g2 (folded into the exp bias)
NJJ = NJT // 2              # 16 j-pair blocks for DoubleRow

AF = mybir.ActivationFunctionType
ALU = mybir.AluOpType


def _enable_ldw_opt():
    # walrus defaults to --enable-ldw-opt=false; with it off every LDWEIGHTS
    # serializes against the previous matmul (~427ns vs ~213ns per 512-col
    # matmul).  Patch the arg builder so the stationary loads pipeline.
    import concourse.bass_utils as _bu
    if getattr(_bu, "_ldw_opt_patched", False):
        return
    _orig = _bu.get_walrus_args

    def _patched(*a, **k):
        args = _orig(*a, **k)
        return [x.replace("--enable-ldw-opt=false", "--enable-ldw-opt=true")
                for x in args]

    _bu.get_walrus_args = _patched
    _bu._ldw_opt_patched = True


def build():
    dt = mybir.dt
    _enable_ldw_opt()
    nc = bacc.Bacc(num_devices=NCORES)

    # DR-interleaved fp8 adjacency: [half, p, jj, i, n] flattened so each
    # partition's (jj, i, n) block is one contiguous 8KB run
    adj8_d = nc.dram_tensor("adj8", [2 * 128, 8 * 2 * SLAB], dt.float8e4,
                            kind="ExternalInput")
    xTh_d = nc.dram_tensor("xT_hi", [128, NFT * SLAB], dt.bfloat16,
                           kind="ExternalInput")
    U6_d = nc.dram_tensor("U6", [F, 8], dt.bfloat16, kind="ExternalInput")
    # negC[p, h] = -max_i s2_i(head h), host-computed and pre-broadcast
    negC_d = nc.dram_tensor("negC", [128, NH], dt.float32,
                            kind="ExternalInput")
    W_d = nc.dram_tensor("W", [128, NH * NFT * HID], dt.bfloat16,
                         kind="ExternalInput")
    ident_d = nc.dram_tensor("ident", [128, 128], dt.bfloat16,
                             kind="ExternalInput")
    Wo_d = nc.dram_tensor("Wo", [128, NCT * G2C], dt.float8e4,
                          kind="ExternalInput")
    out_d = nc.dram_tensor("out", [SLAB, NCLS], dt.float32,
                           kind="ExternalOutput")

    # DRAM scratch + collective buffers.  Per-core staging row order is
    # [p, q, i, c]: partition-major so both staging writes and gathered
    # reloads move multi-hundred-byte contiguous runs.  A gathered core
    # block is gf[jb*128:(jb+1)*128, :] — one fully contiguous reload.
    gsW = nc.dram_tensor("gsW", [128, 4 * WCOLS], dt.float8e4)
    gfW = nc.dram_tensor("gfW", [N // 4, 4 * WCOLS], dt.float8e4,
                         addr_space="Shared")
    gs0 = nc.dram_tensor("gs0", [128, 4 * HID], dt.float8e4)
    gf0 = nc.dram_tensor("gf0", [N // 4, 4 * HID], dt.float8e4,
                         addr_space="Shared")
    gs12 = [nc.dram_tensor(f"gs{h}", [128, 4 * HID], dt.float8e4)
            for h in (1, 2)]
    gf12 = [nc.dram_tensor(f"gf{h}", [N // 4, 4 * HID], dt.float8e4,
                           addr_space="Shared") for h in (1, 2)]
    g2s = nc.dram_tensor("g2s", [128, 4 * C264], dt.float8e4)
    g2f = nc.dram_tensor("g2f", [N // 4, 4 * C264], dt.float8e4,
                         addr_space="Shared")

    rg = [list(range(NCORES))]
    DR = mybir.MatmulPerfMode.DoubleRow

    with tile.TileContext(nc) as tc:
      with (
          tc.tile_pool(name="adjt", bufs=2) as p_adjt,
          tc.tile_pool(name="wo", bufs=1) as p_wo,
      ):
        # fp8 adjacency, j-pair interleaved for DoubleRow (both layers)
        adj8_all = []
        adj8_t = adj8_d.rearrange("(half p) (jj i n) -> half p jj i n",
                                  half=2, jj=NJJ // 2, i=2)

        def adjd(jj):
            return adj8_all[jj // (NJJ // 2)][:, jj % (NJJ // 2), :, :]

        # ---------------- phase 1: s2, w, h=x@W, G build + gathers ----------
        with (
            tc.tile_pool(name="xw", bufs=1) as p_xw,
            tc.tile_pool(name="small", bufs=1) as p_sm,
            tc.tile_pool(name="gtmp", bufs=1) as p_gt,
        ):
            u6 = p_sm.tile([128, NFT, 8], dt.bfloat16, tag="u6", name="u6")
            nc.gpsimd.dma_start(u6[:], U6_d.rearrange("(ft p) c -> p ft c",
                                                      p=128))
            negCbc = p_sm.tile([128, NH], dt.float32, tag="negCbc",
                               name="negCbc")
            nc.gpsimd.dma_start(negCbc[:], negC_d[:])

            # x and W: layouts are host-pre-packed so each DMA is one
            # contiguous run per partition
            xh_all = p_xw.tile([128, NFT, SLAB], dt.bfloat16, tag="xh",
                               name="xh")
            nc.sync.dma_start(xh_all[:], xTh_d.rearrange("p (ft i) -> p ft i",
                                                         ft=NFT))

            def xhi(ft, c0, c1):
                return xh_all[:, ft, c0:c1]

            W_t = W_d.rearrange("p (h ft o) -> p h ft o", h=NH, ft=NFT)
            w0_all = p_xw.tile([128, NFT, HID], dt.bfloat16, tag="w0",
                               name="w0")
            nc.sync.dma_start(w0_all[:], W_t[:, 0])
            w12_all = p_xw.tile([128, 2, NFT, HID], dt.bfloat16, tag="w12",
                                name="w12")
            nc.scalar.dma_start(w12_all[:], W_t[:, 1:3])

            def wsl(h, ft, c0, c1):
                if h == 0:
                    return w0_all[:, ft, c0:c1]
                return w12_all[:, h - 1, ft, c0:c1]

            # Wo early on scalar (needed from the first L1 epilogue)
            wo_all = p_wo.tile([128, NCT // 2, 2, G2C], dt.float8e4,
                               tag="wo", name="wo")
            nc.scalar.dma_start(wo_all[:],
                                Wo_d.rearrange("p (t d c) -> p t d c",
                                               t=NCT // 2, d=2))
            ident = p_wo.tile([128, 128], dt.bfloat16, tag="ident",
                              name="ident")
            nc.scalar.dma_start(ident[:], ident_d[:])

            # adjacency halves on gpsimd (2MB total, big runs; gpsimd
            # only posts descriptors so later triggers aren't held up)
            for half in range(2):
                t = p_adjt.tile([128, NJJ // 2, 2, SLAB], dt.float8e4,
                                tag="adj8", name="adj8", bufs=2)
                nc.gpsimd.dma_start(t[:], adj8_t[half])
                adj8_all.append(t)

            # s2 = x_hi @ (u_hi + u_lo) FIRST: the whole w chain rides the
            # DVE/Scalar engines under the head-0 x@W that follows.
            s2_sb = []
            for h in range(NH):
                s2_sb.append(p_sm.tile([128, NIT], dt.float32, tag="s2",
                                       name="s2", bufs=NH))
            with tc.tile_pool(name="psS", bufs=1, space="PSUM") as ps_s:
                p6 = ps_s.tile([128, NIT, 8], dt.float32, tag="p6", name="p6")
                for it in range(NIT):
                    for ft in range(NFT):
                        xh = xhi(ft, it * 128, (it + 1) * 128)
                        nc.tensor.matmul(p6[:, it, :], xh, u6[:, ft, :],
                                         start=(ft == 0), stop=(ft == NFT - 1))
                for it in range(NIT):
                    t6 = p_sm.tile([128, 8], dt.float32, tag="t6", name="t6",
                                   bufs=2)
                    nc.vector.tensor_copy(t6[:], p6[:, it, :])
                    tsum = p_sm.tile([128, NH], dt.float32, tag="tsum",
                                     name="tsum", bufs=2)
                    nc.vector.tensor_tensor(tsum[:], t6[:, 0:2 * NH:2],
                                            t6[:, 1:2 * NH:2], ALU.add)
                    for h in range(NH):
                        nc.vector.tensor_copy(s2_sb[h][:, it:it + 1],
                                              tsum[:, h:h + 1])

            # w = exp(s2 - C) with the host-computed C.  Stage w*SW as an
            # fp8 hi+lo pair (lo unscaled: both matmuls then accumulate into
            # one psum against the same adjacency operand), and keep w*SG in
            # fp32 for scaling G.
            w8_sb = []
            whi3 = p_sm.tile([128, NH, NIT], dt.float8e4, tag="whi3",
                             name="whi3")
            wlo3 = p_sm.tile([128, NH, NIT], dt.float8e4, tag="wlo3",
                             name="wlo3")
            for h in range(NH):
                w = p_sm.tile([128, NIT], dt.float32, tag="wexp", name="wexp",
                              bufs=2)
                nc.scalar.activation(w[:], s2_sb[h][:], AF.Exp,
                                     bias=negCbc[:, h:h + 1])
                w8 = p_sm.tile([128, NIT], dt.float32, tag="wsg", name="wsg",
                               bufs=NH)
                nc.vector.tensor_scalar_mul(w8[:], w[:], SG)
                w8_sb.append(w8)
                wsw = p_sm.tile([128, NIT], dt.float32, tag="wsw", name="wsw",
                                bufs=2)
                nc.vector.tensor_scalar_mul(wsw[:], w[:], SW)
                nc.vector.tensor_copy(whi3[:, h, :], wsw[:])
                wr = p_sm.tile([128, NIT], dt.float32, tag="wr", name="wr",
                               bufs=2)
                nc.vector.tensor_tensor(wr[:], wsw[:], whi3[:, h, :],
                                        ALU.subtract)
                nc.vector.tensor_copy(wlo3[:, h, :], wr[:])

            # stage w (hi|lo fp8, 32-col block) and gather it FIRST: the tiny
            # AG absorbs the CC wind-up and unlocks the denominator matmuls
            # long before the big G gathers land
            gsWv = gsW.rearrange("p (q i c) -> p q i c", q=2, i=2)
            for it in range(NIT):
                wt = p_gt.tile([128, WCOLS], dt.float8e4, tag="wt",
                               name="wt", bufs=2)
                nc.vector.memset(wt[:, 2 * NH:WCOLS], 0.0)
                nc.vector.tensor_copy(wt[:, 0:NH], whi3[:, :, it])
                nc.vector.tensor_copy(wt[:, NH:2 * NH], wlo3[:, :, it])
                nc.sync.dma_start(gsWv[:, it // 2, it % 2, :], wt[:])
            nc.gpsimd.collective_compute(
                "AllGather", ALU.bypass, replica_groups=rg,
                ins=[gsW[:]], outs=[gfW[:]])

            # per head: x@W -> G staging -> AllGather, heads ordered (1,2,0)
            # so the merged h1+h2 gather starts early and h0 overlaps it.
            # Staging row order [p, q, i, c] with (q, i) = (it//2, it%2)
            # matches the DR-interleaved adjacency node order.
            gs0v = gs0.rearrange("p (q i c) -> p q i c", q=2, i=2)
            gs12v = [g.rearrange("p (q i c) -> p q i c", q=2, i=2)
                     for g in gs12]
            ctx_psA = tc.tile_pool(name="psA", bufs=3, space="PSUM")
            ps_a = ctx_psA.__enter__()

            def xw_head(h, it):
                ps = ps_a.tile([128, HID], dt.float32, tag="psA", name="psA")
                for ft in range(NFT):
                    xh = xhi(ft, it * 128, (it + 1) * 128)
                    nc.tensor.matmul(ps[:, 0:512], xh, wsl(h, ft, 0, 512),
                                     start=(ft == 0), stop=(ft == NFT - 1))
                    nc.tensor.matmul(ps[:, 512:HID], xh, wsl(h, ft, 512, HID),
                                     start=(ft == 0), stop=(ft == NFT - 1))
                return ps

            for h in (1, 2, 0):
                for it in range(NIT):
                    ps = xw_head(h, it)
                    g = p_gt.tile([128, HID], dt.float8e4, tag="g12",
                                  name="g12", bufs=4)
                    nc.vector.tensor_scalar_mul(g[:], ps[:],
                                                w8_sb[h][:, it:it + 1])
                    # per-head staging queues: keeps each gather trigger's
                    # DMA-completion semaphores free of later heads' DMAs
                    if h == 0:
                        nc.sync.dma_start(gs0v[:, it // 2, it % 2, :], g[:])
                    else:
                        eng = nc.scalar if h == 1 else nc.gpsimd
                        eng.dma_start(gs12v[h - 1][:, it // 2, it % 2, :],
                                      g[:])
                if h == 0:
                    nc.gpsimd.collective_compute(
                        "AllGather", ALU.bypass, replica_groups=rg,
                        ins=[gs0[:]], outs=[gf0[:]])
                else:
                    nc.gpsimd.collective_compute(
                        "AllGather", ALU.bypass, replica_groups=rg,
                        ins=[gs12[h - 1][:]], outs=[gf12[h - 1][:]])
            ctx_psA.__exit__(None, None, None)

        # ---------------- L1 adjacency matmul + epilogue + layer 2 ----------
        # Flipped orientation: the adjacency j-pair slice is the STATIONARY
        # operand and the gathered G rows are the moving operand, so the
        # output lands [i-part, features] and the denominator reciprocal is a
        # per-partition scalar.  The den matmuls run in a cheap standalone
        # phase against the tiny w gather while the big G gathers are still
        # in flight.  h2 needs xcat f-major, restored per (h, it) via six
        # PE transposes batched through one psum bank.
        with tc.tile_pool(name="rcp", bufs=1) as p_rc:
            recip_it = [None] * NIT
            with (
                tc.tile_pool(name="wden", bufs=1) as p_wd,
                tc.tile_pool(name="psd", bufs=1, space="PSUM") as ps_d,
            ):
                gvW = gfW.rearrange("(jb p) (q i c) -> jb p q i c",
                                    p=128, q=2, i=2)
                wts = []
                for jb in range(NCORES):
                    wt = p_wd.tile([128, 2, 2, WCOLS], dt.float8e4,
                                   tag="wtg", name="wtg", bufs=NCORES)
                    eng = nc.sync if jb % 2 == 0 else nc.scalar
                    eng.dma_start(wt[:], gvW[jb])
                    wts.append(wt)
                pd = ps_d.tile([128, NIT, 8], dt.float32, tag="pd",
                               name="pd")
                for it in range(NIT):
                    for jb in range(NCORES):
                        for q in range(2):
                            jj = 2 * jb + q
                            lhs = adjd(jj)[:, :, it * 128:(it + 1) * 128]
                            nc.tensor.matmul(pd[:, it, 0:2 * NH], lhs,
                                             wts[jb][:, q, :, 0:2 * NH],
                                             start=(jj == 0),
                                             stop=(jj == NJJ - 1),
                                             perf_mode=DR)
                for it in range(NIT):
                    den6 = p_rc.tile([128, 2 * NH], dt.float32, tag="den6",
                                     name="den6", bufs=2)
                    nc.vector.tensor_copy(den6[:], pd[:, it, 0:2 * NH])
                    dsum = p_rc.tile([128, NH], dt.float32, tag="dsum",
                                     name="dsum", bufs=2)
                    nc.vector.tensor_tensor(dsum[:], den6[:, 0:NH],
                                            den6[:, NH:2 * NH], ALU.add)
                    rc = p_rc.tile([128, NH], dt.float32, tag="rc",
                                   name="rc", bufs=NIT)
                    nc.vector.reciprocal(rc[:], dsum[:])
                    nc.vector.tensor_scalar_mul(rc[:], rc[:], SW / SG)
                    recip_it[it] = rc
            with (
                tc.tile_pool(name="gst", bufs=1) as p_gst,
                tc.tile_pool(name="xct", bufs=1) as p_xct,
                tc.tile_pool(name="etmp", bufs=1) as p_et,
                tc.tile_pool(name="l2a", bufs=1) as p_l2a,
                tc.tile_pool(name="ps1", bufs=1, space="PSUM") as ps_1,
                tc.tile_pool(name="psh2", bufs=4, space="PSUM") as ps_h2,
            ):
                gv0 = gf0.rearrange("(jb p) (q i c) -> jb p q i c",
                                    p=128, q=2, i=2)
                gv12 = [g.rearrange("(jb p) (q i c) -> jb p q i c",
                                    p=128, q=2, i=2) for g in gf12]
                ps2l = [ps_h2.tile([128, G2C], dt.float32, tag="psh2",
                                   name="psh2") for _ in range(NIT)]
                nct_seen = [0] * NIT

                # transpose + h2 accumulation for one finished (h, it),
                # queued one step late so the PE never waits on the DVE
                # epilogue
                def h2_block(h, it, xc):
                    # all 6 transposes back-to-back into one psum bank, ONE
                    # DVE copy out, then the 6 h2 matmuls
                    pT = ps_1.tile([128, NFT, 128], dt.bfloat16,
                                   tag="pT", name="pT", bufs=2)
                    for fb in range(NFT):
                        nc.tensor.transpose(pT[:, fb, :],
                                            xc[:, fb * 128:(fb + 1) * 128],
                                            ident[:])
                    xcT = p_xct.tile([128, NFT, 128], dt.float8e4,
                                     tag="xcT", name="xcT", bufs=2)
                    nc.vector.tensor_copy(xcT[:], pT[:])
                    for cp2 in range(NFT // 2):
                        tp = h * (NFT // 2) + cp2
                        n = nct_seen[it]
                        nct_seen[it] += 1
                        nc.tensor.matmul(ps2l[it][:],
                                         xcT[:, 2 * cp2:2 * cp2 + 2, :],
                                         wo_all[:, tp, :, :],
                                         start=(n == 0),
                                         stop=(n == NCT // 2 - 1),
                                         perf_mode=DR)

                pending = []
                for h in (1, 2, 0):
                    # drain the deferred-work backlog before this head's
                    # first matmul can stall on its gather
                    while len(pending) > 1:
                        h2_block(*pending.pop(0))
                    gts = []
                    for jb in range(NCORES):
                        gt = p_gst.tile([128, 2, 2, HID], dt.float8e4,
                                        tag="gt", name="gt", bufs=10)
                        eng = nc.sync if jb % 2 == 0 else nc.scalar
                        src = gv0[jb] if h == 0 else gv12[h - 1][jb]
                        eng.dma_start(gt[:], src)
                        gts.append(gt)
                    for it in range(NIT):
                        pg = ps_1.tile([128, HID], dt.float32, tag="pg",
                                       name="pg", bufs=1)
                        for jb in range(NCORES):
                            gt = gts[jb]
                            for q in range(2):
                                jj = 2 * jb + q
                                lhs = adjd(jj)[:, :,
                                               it * 128:(it + 1) * 128]
                                nc.tensor.matmul(pg[:, 0:512], lhs,
                                                 gt[:, q, :, 0:512],
                                                 start=(jj == 0),
                                                 stop=(jj == NJJ - 1),
                                                 perf_mode=DR)
                                nc.tensor.matmul(pg[:, 512:HID], lhs,
                                                 gt[:, q, :, 512:HID],
                                                 start=(jj == 0),
                                                 stop=(jj == NJJ - 1),
                                                 perf_mode=DR)
                        # one copy releases the single psum buffer fast; the
                        # epilogue reads the SBUF copy
                        pgc = p_et.tile([128, HID], dt.float32, tag="pgc",
                                        name="pgc", bufs=2)
                        nc.vector.tensor_copy(pgc[:], pg[:])
                        # xcat i-tile = elu(num / den), bf16 [128 i, 768 f]
                        z = p_et.tile([128, HID], dt.float32, tag="z",
                                      name="z", bufs=2)
                        nc.vector.tensor_scalar_mul(z[:], pgc[:],
                                                    recip_it[it][:,
                                                                 h:h + 1])
                        e = p_et.tile([128, HID], dt.float32, tag="e",
                                      name="e", bufs=2)
                        nc.scalar.activation(e[:], z[:], AF.Exp)
                        nc.vector.tensor_scalar(e[:], e[:], 1.0, -1.0,
                                                ALU.min, ALU.add)
                        xc = p_xct.tile([128, HID], dt.bfloat16,
                                        tag="xcp", name="xcp", bufs=5)
                        nc.vector.scalar_tensor_tensor(xc[:], z[:], 0.0,
                                                       e[:], ALU.max,
                                                       ALU.add)
                        # keep a backlog of deferred transpose+h2 work so
                        # the PE has something to chew on while the next
                        # head's gather lands
                        if len(pending) >= 3:
                            h2_block(*pending.pop(0))
                        pending.append((h, it, xc))
                while pending:
                    h2_block(*pending.pop(0))

                # layer-2 g2 = [w2*h2 | w2] as fp8 (hi only); the fp8 scale
                # S2 rides the exp bias (a global scale cancels in num/den)
                # and NO max subtraction is needed (s2' stays O(1)).
                lnS2 = p_l2a.tile([128, 1], dt.float32, tag="lnS2",
                                  name="lnS2")
                nc.vector.memset(lnS2[:], float(np.log(S2)))
                g2sv = g2s.rearrange("p (q i c) -> p q i c", q=2, i=2)
                ghl = p_l2a.tile([128, 2, 2, C264], dt.float8e4, tag="ghl",
                                 name="ghl")
                for it in range(NIT):
                    q, i = it // 2, it % 2
                    w2 = p_l2a.tile([128, 1], dt.float32, tag="w2",
                                    name="w2", bufs=2)
                    nc.scalar.activation(w2[:], ps2l[it][:, NCLS:G2C],
                                         AF.Exp, bias=lnS2[:])
                    nc.vector.tensor_scalar_mul(ghl[:, q, i, 0:NCLS],
                                                ps2l[it][:, 0:NCLS], w2[:])
                    nc.vector.tensor_copy(ghl[:, q, i, NCLS:G2C], w2[:])
                    nc.vector.memset(ghl[:, q, i, G2C:C264], 0.0)
                    nc.sync.dma_start(g2sv[:, q, i, :], ghl[:, q, i, :])
                nc.gpsimd.collective_compute(
                    "AllGather", ALU.bypass, replica_groups=rg,
                    ins=[g2s[:]], outs=[g2f[:]])

        # ---------------- L2 adjacency matmul (fp8 DR) + final epilogue ----
        with (
            tc.tile_pool(name="g2t", bufs=1) as p_g2t,
            tc.tile_pool(name="fin", bufs=1) as p_f,
            tc.tile_pool(name="ps2", bufs=1, space="PSUM") as ps_2,
        ):
            ps2 = ps_2.tile([128, NIT, 512], dt.float32, tag="ps2",
                            name="ps2", bufs=1)
            g2v = g2f.rearrange("(jb p) (q i c) -> jb p q i c",
                                p=128, q=2, i=2)
            g2tiles = []
            for jb in range(NCORES):
                gt2 = p_g2t.tile([128, 2, 2, C264], dt.float8e4,
                                 tag="g2t", name="g2t", bufs=NCORES)
                eng = nc.sync if jb % 2 == 0 else nc.scalar
                eng.dma_start(gt2[:], g2v[jb])
                g2tiles.append(gt2)
            if True:
                for it in range(NIT):
                    for jb in range(NCORES):
                        for k in range(2):
                            jj = 2 * jb + k
                            lhs = adjd(jj)[:, :, it * 128:(it + 1) * 128]
                            nc.tensor.matmul(
                                ps2[:, it, 0:C264], lhs,
                                g2tiles[jb][:, k, :, :],
                                start=(jb == 0 and k == 0),
                                stop=(jb == NCORES - 1 and k == 1),
                                perf_mode=DR)
                # inline epilogue per i-tile, spread across engines so the
                # serial tail after the last matmul stays short
                r2 = p_f.tile([128, 1], dt.float32, tag="r2", name="r2",
                              bufs=2)
                nc.vector.reciprocal(r2[:], ps2[:, it, NCLS:G2C])
                z = p_f.tile([128, NCLS], dt.float32, tag="z2", name="z2",
                             bufs=2)
                nc.vector.tensor_scalar_mul(z[:], ps2[:, it, 0:NCLS], r2[:])
                e = p_f.tile([128, NCLS], dt.float32, tag="e2", name="e2",
                             bufs=2)
                nc.scalar.activation(e[:], z[:], AF.Exp)
                nc.vector.tensor_scalar(e[:], e[:], 1.0, -1.0, ALU.min,
                                        ALU.add)
                o = p_f.tile([128, NCLS], dt.float32, tag="o2", name="o2",
                             bufs=2)
                nc.vector.scalar_tensor_tensor(o[:], z[:], 0.0, e[:],
                                               ALU.max, ALU.add)
                # log_softmax without max subtraction (o <= ~10)
                t4 = p_f.tile([128, NCLS], dt.float32, tag="t4", name="t4",
                              bufs=2)
                ssum = p_f.tile([128, 1], dt.float32, tag="ssum",
                                name="ssum", bufs=2)
                nc.scalar.activation(t4[:], o[:], AF.Exp, accum_out=ssum[:])
                lg = p_f.tile([128, 1], dt.float32, tag="lg", name="lg",
                              bufs=2)
                nc.scalar.activation(lg[:], ssum[:], AF.Ln)
                fin = p_f.tile([128, NCLS], dt.float32, tag="fin",
                               name="fin", bufs=2)
                nc.vector.tensor_scalar(fin[:], o[:], lg[:], None,
                                        ALU.subtract)
                nc.sync.dma_start(out_d[it * 128:(it + 1) * 128, :], fin[:])

    nc.finalize()
    return nc


_CACHE = {}


def _pair(a):
    hi = a.astype(BF16)
    lo = (a - hi.astype(np.float32)).astype(BF16)
    return hi, lo


def prepare_inputs(x, adj, W_heads, a_heads, W_out, a_out):
    """Shard + lay out the full inputs for the 8 cores."""
    x2 = np.asarray(x, np.float32)[0]          # [N, F]
    adj2 = np.asarray(adj)[0]                  # [N, N] int32
    W3 = np.asarray(W_heads, np.float32).reshape(NH, F, HID)
    a3 = np.asarray(a_heads, np.float32)       # [NH, 2*HID, 1]
    Wo = np.asarray(W_out, np.float32).reshape(GH_TOT, NCLS)
    ao = np.asarray(a_out, np.float32)         # [2*NCLS, 1]

    # fold the edge-score projections into the weights:
    #   s2 = x @ (W @ a2),   s2' = xcat @ (Wo @ ao2)
    u = np.einsum("hfo,ho->hf", W3.astype(np.float64),
                  a3[:, HID:, 0].astype(np.float64)).astype(np.float32)
    u_hi, u_lo = _pair(u)
    U6 = np.zeros((F, 8), BF16)
    for h in range(NH):
        U6[:, 2 * h] = u_hi[h]
        U6[:, 2 * h + 1] = u_lo[h]
    u2 = (Wo.astype(np.float64)
          @ ao[NCLS:, 0].astype(np.float64)).astype(np.float32)
    Wo_ext = np.concatenate([Wo, u2[:, None]], axis=1)       # [GH, 257]
    # [p, ct, c] packed so the Wo DMA is one contiguous run per partition
    Wo_b = np.ascontiguousarray(
        Wo_ext.astype(F8E4).reshape(NCT // 2, 2, 128, G2C)
        .transpose(2, 0, 1, 3)
    ).reshape(128, NCT * G2C)
    # [p, h, ft, o] packed W
    W_b = np.ascontiguousarray(
        W3.astype(BF16).reshape(NH, NFT, 128, HID).transpose(2, 0, 1, 3)
    ).reshape(128, NH * NFT * HID)
    xT = np.ascontiguousarray(x2.T)            # [F, N]
    adj8 = (adj2 > 0).astype(F8E4)             # exact 0/1

    # exact per-head max of s2 = x @ u, folded on the host so the device
    # needs no max-reduction collective.  Mirror the device arithmetic
    # (bf16 x_hi against the u hi/lo pair, accumulated in fp32).
    xh_f = x2.astype(BF16).astype(np.float32)
    s2 = (xh_f @ u_hi.T.astype(np.float32)
          + xh_f @ u_lo.T.astype(np.float32))                     # [N, NH]
    negC = np.broadcast_to(
        -s2.max(axis=0, keepdims=True), (128, NH)
    ).astype(np.float32)                                          # [128, NH]

    in_maps = []
    for c in range(NCORES):
        sl = slice(c * SLAB, (c + 1) * SLAB)
        # [p, ft, i] packed x slab
        xh = np.ascontiguousarray(
            xT[:, sl].astype(BF16).reshape(NFT, 128, SLAB).transpose(1, 0, 2)
        ).reshape(128, NFT * SLAB)
        # DR-interleaved adjacency: [half, p, jj, i, n] with j decomposed as
        # half*2048 + jj*256 + i*128 + p
        adjTc = np.ascontiguousarray(adj8[sl, :].T)          # [N, SLAB]
        adj8i = np.ascontiguousarray(
            adjTc.reshape(2, NJJ // 2, 2, 128, SLAB).transpose(0, 3, 1, 2, 4)
        ).reshape(2 * 128, NJJ // 2 * 2 * SLAB)
        in_maps.append({
            "adj8": adj8i,
            "xT_hi": xh,
            "U6": U6, "negC": negC,
            "W": W_b, "Wo": Wo_b,
            "ident": np.eye(128, dtype=BF16),
        })
    return in_maps


def kernel(x, adj, W_heads, a_heads, W_out, a_out):
    if "nc" not in _CACHE:
        # touch the devices once so any residual bad state from a previous
        # process surfaces (and clears) before the real run
        try:
            import jax
            jax.block_until_ready(jax.numpy.zeros(8))
        except Exception:
            pass
        _CACHE["nc"] = build()
    nc = _CACHE["nc"]
    in_maps = prepare_inputs(x, adj, W_heads, a_heads, W_out, a_out)
    res = run_bass_kernel_spmd(nc, in_maps, list(range(NCORES)))
    out = np.concatenate([res.results[c]["out"] for c in range(NCORES)],
                         axis=0)
    return out.reshape(1, N, NCLS)


# revision 61
# speedup vs baseline: 1.0397x; 1.0397x over previous
"""GAT (2-layer, 3-head) forward on 8 Trainium2 NeuronCores.

Math: with LeakyReLU slope ALPHA=1.0 the edge score e_ij = s1_i + s2_j is
linear, and s1_i cancels inside the row softmax.  The masked softmax over
j therefore reduces to column weights w_j = exp(s2_j - C) restricted to
adj, giving

    h'_i = (sum_j adj_ij * w_j * h_j) / (sum_j adj_ij * w_j)

i.e. one adjacency matmul against G = [w*h | w].  Both GAT layers take
this form (the same adjacency masks both), so the whole network is two
A-matmuls plus small projections.

Sharding: rows of h' (nodes) across 8 cores; each core holds the fp8
DoubleRow-interleaved adjacency columns for its 512-row slab and
computes the slab.

Precision (tolerance 2e-2, achieved ~8e-3): the adjacency matmuls of
BOTH layers and the h2 projection run in fp8 e4m3 with DoubleRow perf
mode (each matmul contracts a 256-row pair at 2x rate).  Layer 1: G
scaled by 8; the denominator weights w by 128 as an fp8 hi+lo pair (lo
stored unscaled so hi and lo accumulate into one psum against the same
adjacency operand).  Layer 2: g2 = [w2*h2 | w2] in fp8 with the scale
folded into the exp bias (a global scale cancels in num/den).  x@W is
bf16; s2 uses a bf16 hi/lo pair of the folded u vector; its softmax max
is computed exactly on the host (negC input, pre-broadcast); layer 2
skips max subtraction (s2' stays O(1)).

Schedule: all DRAM layouts are pre-interleaved host-side so every DMA
moves multi-KB contiguous runs per partition, and the adjacency is
loaded once in the DR-interleaved fp8 form both layers consume.  The
L1 matmuls run with the adjacency slice STATIONARY, so the output lands
[i-part, features] and the softmax reciprocal is a per-partition scalar.
s2/w are computed first and the tiny w gather goes out ahead of the big
ones — it absorbs the CC-stream wind-up (~55us fixed bring-up + ~10us
first-op delay) and feeds a standalone denominator-matmul phase that
runs while the G gathers are still in flight.  Heads are processed
(1, 2, 0) with per-head AllGathers (per-head staging queues keep each
trigger's DMA-completion semaphores free of later heads' traffic).
xcat returns to f-major for h2 via six PE transposes batched through
one psum bank, deferred one step so the PE never waits on the DVE
epilogue; h2 accumulates via fp8 DoubleRow against host-pair-interleaved
Wo.  g2 is gathered once and the elu+log_softmax epilogue is inlined
per i-tile so it overlaps the remaining L2 matmuls.
"""
import sys

sys.path.insert(0, "/opt/trn_rl_repo")

import numpy as np
import ml_dtypes

import concourse.bass as bass
import concourse.bacc as bacc
import concourse.mybir as mybir
import concourse.bass_isa as bass_isa
import concourse.tile as tile
from concourse.bass_utils import run_bass_kernel_spmd

BF16 = ml_dtypes.bfloat16
F8E4 = ml_dtypes.float8_e4m3

N = 4096
F = 768
HID = 768
NH = 3
NCLS = 256
NCORES = 8
SLAB = N // NCORES          # 512 rows per core
NIT = SLAB // 128           # 4 i-tiles per core
NJT = N // 128              # 32 j-tiles
NFT = F // 128              # 6 f-tiles
NCT = NH * NFT              # 18 feature col-tiles of G
G2C = NCLS + 1              # 257 = classes + s2' column (folded u2)
C264 = 264                  # fp8 g2 row padded to 264
WCOLS = 32                  # w-column block (6 used + pad)
GA = WCOLS + HID            # gather-0 row bytes: [w cols | head0 G]
GH_TOT = NH * HID           # 2304 xcat feature rows of Wo
SG = 8.0                    # fp8 scale on G ( |G*8| << 240 )
SW = 128.0                  # fp8 scale on w (w <= 1)
S2 = 32.0                   # fp8 scale on g2 (folded into the exp bias)
NJJ = NJT // 2              # 16 j-pair blocks for DoubleRow

AF = mybir.ActivationFunctionType
ALU = mybir.AluOpType


def _enable_ldw_opt():
    # walrus defaults to --enable-ldw-opt=false; with it off every LDWEIGHTS
    # serializes against the previous matmul (~427ns vs ~213ns per 512-col
    # matmul).  Patch the arg builder so the stationary loads pipeline.
    import concourse.bass_utils as _bu
    if getattr(_bu, "_ldw_opt_patched", False):
        return
    _orig = _bu.get_walrus_args

    def _patched(*a, **k):
        args = _orig(*a, **k)
        return [x.replace("--enable-ldw-opt=false", "--enable-ldw-opt=true")
                for x in args]

    _bu.get_walrus_args = _patched
    _bu._ldw_opt_patched = True


def build():
    dt = mybir.dt
    _enable_ldw_opt()
    nc = bacc.Bacc(num_devices=NCORES)

    # DR-interleaved fp8 adjacency: [half, p, jj, i, n] flattened so each
    # partition's (jj, i, n) block is one contiguous 8KB run
    adj8_d = nc.dram_tensor("adj8", [2 * 128, 8 * 2 * SLAB], dt.float8e4,
                            kind="ExternalInput")
    xTh_d = nc.dram_tensor("xT_hi", [128, NFT * SLAB], dt.bfloat16,
                           kind="ExternalInput")
    U6_d = nc.dram_tensor("U6", [F, 8], dt.bfloat16, kind="ExternalInput")
    # negC[p, h] = -max_i s2_i(head h), host-computed and pre-broadcast
    negC_d = nc.dram_tensor("negC", [128, NH], dt.float32,
                            kind="ExternalInput")
    W_d = nc.dram_tensor("W", [128, NH * NFT * HID], dt.bfloat16,
                         kind="ExternalInput")
    ident_d = nc.dram_tensor("ident", [128, 128], dt.bfloat16,
                             kind="ExternalInput")
    Wo_d = nc.dram_tensor("Wo", [128, NCT * G2C], dt.float8e4,
                          kind="ExternalInput")
    out_d = nc.dram_tensor("out", [SLAB, NCLS], dt.float32,
                           kind="ExternalOutput")

    # DRAM scratch + collective buffers.  Per-core staging row order is
    # [p, q, i, c]: partition-major so both staging writes and gathered
    # reloads move multi-hundred-byte contiguous runs.  A gathered core
    # block is gf[jb*128:(jb+1)*128, :] — one fully contiguous reload.
    gs0 = nc.dram_tensor("gs0", [128, 4 * HID], dt.float8e4)
    gf0 = nc.dram_tensor("gf0", [N // 4, 4 * HID], dt.float8e4,
                         addr_space="Shared")
    # head 1 is processed first, so its gather carries the w columns
    # (hi 0:3 | lo 3:6 | pad) inline: the denominator rides its matmuls
    gs12 = [nc.dram_tensor(f"gs{h}", [128, 4 * (GA if h == 1 else HID)],
                           dt.float8e4) for h in (1, 2)]
    gf12 = [nc.dram_tensor(f"gf{h}", [N // 4, 4 * (GA if h == 1 else HID)],
                           dt.float8e4, addr_space="Shared")
            for h in (1, 2)]
    g2s = nc.dram_tensor("g2s", [128, 4 * C264], dt.float8e4)
    g2f = nc.dram_tensor("g2f", [N // 4, 4 * C264], dt.float8e4,
                         addr_space="Shared")

    rg = [list(range(NCORES))]
    DR = mybir.MatmulPerfMode.DoubleRow

    with tile.TileContext(nc) as tc:
      with (
          tc.tile_pool(name="adjt", bufs=2) as p_adjt,
          tc.tile_pool(name="wo", bufs=1) as p_wo,
      ):
        # fp8 adjacency, j-pair interleaved for DoubleRow (both layers)
        adj8_all = []
        adj8_t = adj8_d.rearrange("(half p) (jj i n) -> half p jj i n",
                                  half=2, jj=NJJ // 2, i=2)

        def adjd(jj):
            return adj8_all[jj // (NJJ // 2)][:, jj % (NJJ // 2), :, :]

        # ---------------- phase 1: s2, w, h=x@W, G build + gathers ----------
        with (
            tc.tile_pool(name="xw", bufs=1) as p_xw,
            tc.tile_pool(name="small", bufs=1) as p_sm,
            tc.tile_pool(name="gtmp", bufs=1) as p_gt,
        ):
            u6 = p_sm.tile([128, NFT, 8], dt.bfloat16, tag="u6", name="u6")
            nc.gpsimd.dma_start(u6[:], U6_d.rearrange("(ft p) c -> p ft c",
                                                      p=128))
            negCbc = p_sm.tile([128, NH], dt.float32, tag="negCbc",
                               name="negCbc")
            nc.gpsimd.dma_start(negCbc[:], negC_d[:])

            # x and W: layouts are host-pre-packed so each DMA is one
            # contiguous run per partition
            xh_all = p_xw.tile([128, NFT, SLAB], dt.bfloat16, tag="xh",
                               name="xh")
            nc.sync.dma_start(xh_all[:], xTh_d.rearrange("p (ft i) -> p ft i",
                                                         ft=NFT))

            def xhi(ft, c0, c1):
                return xh_all[:, ft, c0:c1]

            W_t = W_d.rearrange("p (h ft o) -> p h ft o", h=NH, ft=NFT)
            w0_all = p_xw.tile([128, NFT, HID], dt.bfloat16, tag="w0",
                               name="w0")
            nc.sync.dma_start(w0_all[:], W_t[:, 0])
            w12_all = p_xw.tile([128, 2, NFT, HID], dt.bfloat16, tag="w12",
                                name="w12")
            nc.scalar.dma_start(w12_all[:], W_t[:, 1:3])

            def wsl(h, ft, c0, c1):
                if h == 0:
                    return w0_all[:, ft, c0:c1]
                return w12_all[:, h - 1, ft, c0:c1]

            # Wo early on scalar (needed from the first L1 epilogue)
            wo_all = p_wo.tile([128, NCT // 2, 2, G2C], dt.float8e4,
                               tag="wo", name="wo")
            nc.scalar.dma_start(wo_all[:],
                                Wo_d.rearrange("p (t d c) -> p t d c",
                                               t=NCT // 2, d=2))
            ident = p_wo.tile([128, 128], dt.bfloat16, tag="ident",
                              name="ident")
            nc.scalar.dma_start(ident[:], ident_d[:])

            # adjacency halves on gpsimd (2MB total, big runs; gpsimd
            # only posts descriptors so later triggers aren't held up)
            for half in range(2):
                t = p_adjt.tile([128, NJJ // 2, 2, SLAB], dt.float8e4,
                                tag="adj8", name="adj8", bufs=2)
                nc.gpsimd.dma_start(t[:], adj8_t[half])
                adj8_all.append(t)

            # s2 = x_hi @ (u_hi + u_lo) FIRST: the whole w chain rides the
            # DVE/Scalar engines under the head-0 x@W that follows.
            s2_sb = []
            for h in range(NH):
                s2_sb.append(p_sm.tile([128, NIT], dt.float32, tag="s2",
                                       name="s2", bufs=NH))
            with tc.tile_pool(name="psS", bufs=1, space="PSUM") as ps_s:
                p6 = ps_s.tile([128, NIT, 8], dt.float32, tag="p6", name="p6")
                for it in range(NIT):
                    for ft in range(NFT):
                        xh = xhi(ft, it * 128, (it + 1) * 128)
                        nc.tensor.matmul(p6[:, it, :], xh, u6[:, ft, :],
                                         start=(ft == 0), stop=(ft == NFT - 1))
                for it in range(NIT):
                    t6 = p_sm.tile([128, 8], dt.float32, tag="t6", name="t6",
                                   bufs=2)
                    nc.vector.tensor_copy(t6[:], p6[:, it, :])
                    tsum = p_sm.tile([128, NH], dt.float32, tag="tsum",
                                     name="tsum", bufs=2)
                    nc.vector.tensor_tensor(tsum[:], t6[:, 0:2 * NH:2],
                                            t6[:, 1:2 * NH:2], ALU.add)
                    for h in range(NH):
                        nc.vector.tensor_copy(s2_sb[h][:, it:it + 1],
                                              tsum[:, h:h + 1])

            # w = exp(s2 - C) with the host-computed C.  Stage w*SW as an
            # fp8 hi+lo pair (lo unscaled: both matmuls then accumulate into
            # one psum against the same adjacency operand), and keep w*SG in
            # fp32 for scaling G.
            w8_sb = []
            whi3 = p_sm.tile([128, NH, NIT], dt.float8e4, tag="whi3",
                             name="whi3")
            wlo3 = p_sm.tile([128, NH, NIT], dt.float8e4, tag="wlo3",
                             name="wlo3")
            for h in range(NH):
                w = p_sm.tile([128, NIT], dt.float32, tag="wexp", name="wexp",
                              bufs=2)
                nc.scalar.activation(w[:], s2_sb[h][:], AF.Exp,
                                     bias=negCbc[:, h:h + 1])
                w8 = p_sm.tile([128, NIT], dt.float32, tag="wsg", name="wsg",
                               bufs=NH)
                nc.vector.tensor_scalar_mul(w8[:], w[:], SG)
                w8_sb.append(w8)
                wsw = p_sm.tile([128, NIT], dt.float32, tag="wsw", name="wsw",
                                bufs=2)
                nc.vector.tensor_scalar_mul(wsw[:], w[:], SW)
                nc.vector.tensor_copy(whi3[:, h, :], wsw[:])
                wr = p_sm.tile([128, NIT], dt.float32, tag="wr", name="wr",
                               bufs=2)
                nc.vector.tensor_tensor(wr[:], wsw[:], whi3[:, h, :],
                                        ALU.subtract)
                nc.vector.tensor_copy(wlo3[:, h, :], wr[:])

            # per head: x@W -> G staging -> AllGather, heads ordered (1,2,0)
            # so the merged h1+h2 gather starts early and h0 overlaps it.
            # Staging row order [p, q, i, c] with (q, i) = (it//2, it%2)
            # matches the DR-interleaved adjacency node order.
            gs0v = gs0.rearrange("p (q i c) -> p q i c", q=2, i=2)
            gs12v = [g.rearrange("p (q i c) -> p q i c", q=2, i=2)
                     for g in gs12]  # c = GA for h1, HID for h2
            ctx_psA = tc.tile_pool(name="psA", bufs=3, space="PSUM")
            ps_a = ctx_psA.__enter__()

            def xw_head(h, it):
                ps = ps_a.tile([128, HID], dt.float32, tag="psA", name="psA")
                for ft in range(NFT):
                    xh = xhi(ft, it * 128, (it + 1) * 128)
                    nc.tensor.matmul(ps[:, 0:512], xh, wsl(h, ft, 0, 512),
                                     start=(ft == 0), stop=(ft == NFT - 1))
                    nc.tensor.matmul(ps[:, 512:HID], xh, wsl(h, ft, 512, HID),
                                     start=(ft == 0), stop=(ft == NFT - 1))
                return ps

            for h in (1, 2, 0):
                for it in range(NIT):
                    ps = xw_head(h, it)
                    g = p_gt.tile([128, GA], dt.float8e4, tag="g12",
                                  name="g12", bufs=4)
                    if h == 1:
                        nc.vector.memset(g[:, 2 * NH:WCOLS], 0.0)
                        nc.vector.tensor_copy(g[:, 0:NH], whi3[:, :, it])
                        nc.vector.tensor_copy(g[:, NH:2 * NH],
                                              wlo3[:, :, it])
                        nc.vector.tensor_scalar_mul(
                            g[:, WCOLS:GA], ps[:], w8_sb[h][:, it:it + 1])
                    else:
                        nc.vector.tensor_scalar_mul(
                            g[:, 0:HID], ps[:], w8_sb[h][:, it:it + 1])
                    # per-head staging queues: keeps each gather trigger's
                    # DMA-completion semaphores free of later heads' DMAs
                    if h == 0:
                        nc.sync.dma_start(gs0v[:, it // 2, it % 2, :],
                                          g[:, 0:HID])
                    elif h == 1:
                        nc.scalar.dma_start(
                            gs12v[0][:, it // 2, it % 2, :], g[:])
                    else:
                        nc.gpsimd.dma_start(
                            gs12v[1][:, it // 2, it % 2, :], g[:, 0:HID])
                if h == 0:
                    nc.gpsimd.collective_compute(
                        "AllGather", ALU.bypass, replica_groups=rg,
                        ins=[gs0[:]], outs=[gf0[:]])
                else:
                    nc.gpsimd.collective_compute(
                        "AllGather", ALU.bypass, replica_groups=rg,
                        ins=[gs12[h - 1][:]], outs=[gf12[h - 1][:]])
            ctx_psA.__exit__(None, None, None)

        # ---------------- L1 adjacency matmul + epilogue + layer 2 ----------
        # Flipped orientation: the adjacency j-pair slice is the STATIONARY
        # operand and the gathered G rows are the moving operand, so the
        # output lands [i-part, features] and the denominator reciprocal is a
        # per-partition scalar.  The den matmuls run in a cheap standalone
        # phase against the tiny w gather while the big G gathers are still
        # in flight.  h2 needs xcat f-major, restored per (h, it) via six
        # PE transposes batched through one psum bank.
        with tc.tile_pool(name="rcp", bufs=1) as p_rc:
            recip_it = [None] * NIT
            with (
                tc.tile_pool(name="gst", bufs=1) as p_gst,
                tc.tile_pool(name="xct", bufs=1) as p_xct,
                tc.tile_pool(name="etmp", bufs=1) as p_et,
                tc.tile_pool(name="l2a", bufs=1) as p_l2a,
                tc.tile_pool(name="ps1", bufs=1, space="PSUM") as ps_1,
                tc.tile_pool(name="psh2", bufs=4, space="PSUM") as ps_h2,
            ):
                gv0 = gf0.rearrange("(jb p) (q i c) -> jb p q i c",
                                    p=128, q=2, i=2)
                gv12 = [g.rearrange("(jb p) (q i c) -> jb p q i c",
                                    p=128, q=2, i=2) for g in gf12]
                ps2l = [ps_h2.tile([128, G2C], dt.float32, tag="psh2",
                                   name="psh2") for _ in range(NIT)]
                nct_seen = [0] * NIT

                # transpose + h2 accumulation for one finished (h, it),
                # queued one step late so the PE never waits on the DVE
                # epilogue
                def h2_block(h, it, xc):
                    # all 6 transposes back-to-back into one psum bank, ONE
                    # DVE copy out, then the 6 h2 matmuls
                    pT = ps_1.tile([128, NFT, 128], dt.bfloat16,
                                   tag="pT", name="pT", bufs=2)
                    for fb in range(NFT):
                        nc.tensor.transpose(pT[:, fb, :],
                                            xc[:, fb * 128:(fb + 1) * 128],
                                            ident[:])
                    xcT = p_xct.tile([128, NFT, 128], dt.float8e4,
                                     tag="xcT", name="xcT", bufs=2)
                    nc.vector.tensor_copy(xcT[:], pT[:])
                    for cp2 in range(NFT // 2):
                        tp = h * (NFT // 2) + cp2
                        n = nct_seen[it]
                        nct_seen[it] += 1
                        nc.tensor.matmul(ps2l[it][:],
                                         xcT[:, 2 * cp2:2 * cp2 + 2, :],
                                         wo_all[:, tp, :, :],
                                         start=(n == 0),
                                         stop=(n == NCT // 2 - 1),
                                         perf_mode=DR)

                pending = []
                for h in (1, 2, 0):
                    # drain the deferred-work backlog before this head's
                    # first matmul can stall on its gather
                    while len(pending) > 1:
                        h2_block(*pending.pop(0))
                    gw = GA if h == 1 else HID
                    off = WCOLS if h == 1 else 0
                    gts = []
                    for jb in range(NCORES):
                        gt = p_gst.tile([128, 2, 2, gw], dt.float8e4,
                                        tag=("gt1" if h == 1 else "gt"),
                                        name="gt",
                                        bufs=(8 if h == 1 else 10))
                        eng = nc.sync if jb % 2 == 0 else nc.scalar
                        gsrc = gv0[jb] if h == 0 else gv12[h - 1][jb]
                        eng.dma_start(gt[:], gsrc)
                        gts.append(gt)
                    for it in range(NIT):
                        pg = ps_1.tile([128, GA], dt.float32, tag="pg",
                                       name="pg", bufs=1)
                        for jb in range(NCORES):
                            gt = gts[jb]
                            for q in range(2):
                                jj = 2 * jb + q
                                lhs = adjd(jj)[:, :,
                                               it * 128:(it + 1) * 128]
                                nc.tensor.matmul(pg[:, 0:512], lhs,
                                                 gt[:, q, :, 0:512],
                                                 start=(jj == 0),
                                                 stop=(jj == NJJ - 1),
                                                 perf_mode=DR)
                                nc.tensor.matmul(pg[:, 512:gw], lhs,
                                                 gt[:, q, :, 512:gw],
                                                 start=(jj == 0),
                                                 stop=(jj == NJJ - 1),
                                                 perf_mode=DR)
                        # one copy releases the single psum buffer fast; the
                        # epilogue reads the SBUF copy
                        pgc = p_et.tile([128, GA], dt.float32, tag="pgc",
                                        name="pgc", bufs=2)
                        nc.vector.tensor_copy(pgc[:, 0:gw], pg[:, 0:gw])
                        if h == 1:
                            # den cols rode the matmul: hi 0:3, lo 3:6
                            dsum = p_rc.tile([128, NH], dt.float32,
                                             tag="dsum", name="dsum",
                                             bufs=2)
                            nc.vector.tensor_tensor(dsum[:], pgc[:, 0:NH],
                                                    pgc[:, NH:2 * NH],
                                                    ALU.add)
                            rc = p_rc.tile([128, NH], dt.float32, tag="rc",
                                           name="rc", bufs=NIT)
                            nc.vector.reciprocal(rc[:], dsum[:])
                            nc.vector.tensor_scalar_mul(rc[:], rc[:],
                                                        SW / SG)
                            recip_it[it] = rc
                        # xcat i-tile = elu(num / den), bf16 [128 i, 768 f]
                        z = p_et.tile([128, HID], dt.float32, tag="z",
                                      name="z", bufs=2)
                        nc.vector.tensor_scalar_mul(
                            z[:], pgc[:, off:off + HID],
                            recip_it[it][:, h:h + 1])
                        e = p_et.tile([128, HID], dt.float32, tag="e",
                                      name="e", bufs=2)
                        nc.scalar.activation(e[:], z[:], AF.Exp)
                        nc.vector.tensor_scalar(e[:], e[:], 1.0, -1.0,
                                                ALU.min, ALU.add)
                        xc = p_xct.tile([128, HID], dt.bfloat16,
                                        tag="xcp", name="xcp", bufs=5)
                        nc.vector.scalar_tensor_tensor(xc[:], z[:], 0.0,
                                                       e[:], ALU.max,
                                                       ALU.add)
                        # keep a backlog of deferred transpose+h2 work so
                        # the PE has something to chew on while the next
                        # head's gather lands
                        if len(pending) >= 3:
                            h2_block(*pending.pop(0))
                        pending.append((h, it, xc))
                while pending:
                    h2_block(*pending.pop(0))

                # layer-2 g2 = [w2*h2 | w2] as fp8 (hi only); the fp8 scale
                # S2 rides the exp bias (a global scale cancels in num/den)
                # and NO max subtraction is needed (s2' stays O(1)).
                lnS2 = p_l2a.tile([128, 1], dt.float32, tag="lnS2",
                                  name="lnS2")
                nc.vector.memset(lnS2[:], float(np.log(S2)))
                g2sv = g2s.rearrange("p (q i c) -> p q i c", q=2, i=2)
                ghl = p_l2a.tile([128, 2, 2, C264], dt.float8e4, tag="ghl",
                                 name="ghl")
                for it in range(NIT):
                    q, i = it // 2, it % 2
                    w2 = p_l2a.tile([128, 1], dt.float32, tag="w2",
                                    name="w2", bufs=2)
                    nc.scalar.activation(w2[:], ps2l[it][:, NCLS:G2C],
                                         AF.Exp, bias=lnS2[:])
                    nc.vector.tensor_scalar_mul(ghl[:, q, i, 0:NCLS],
                                                ps2l[it][:, 0:NCLS], w2[:])
                    nc.vector.tensor_copy(ghl[:, q, i, NCLS:G2C], w2[:])
                    nc.vector.memset(ghl[:, q, i, G2C:C264], 0.0)
                    nc.sync.dma_start(g2sv[:, q, i, :], ghl[:, q, i, :])
                nc.gpsimd.collective_compute(
                    "AllGather", ALU.bypass, replica_groups=rg,
                    ins=[g2s[:]], outs=[g2f[:]])

        # ---------------- L2 adjacency matmul (fp8 DR) + final epilogue ----
        with (
            tc.tile_pool(name="g2t", bufs=1) as p_g2t,
            tc.tile_pool(name="fin", bufs=1) as p_f,
            tc.tile_pool(name="ps2", bufs=1, space="PSUM") as ps_2,
        ):
            ps2 = ps_2.tile([128, NIT, 512], dt.float32, tag="ps2",
                            name="ps2", bufs=1)
            g2v = g2f.rearrange("(jb p) (q i c) -> jb p q i c",
                                p=128, q=2, i=2)
            g2tiles = []
            for jb in range(NCORES):
                gt2 = p_g2t.tile([128, 2, 2, C264], dt.float8e4,
                                 tag="g2t", name="g2t", bufs=NCORES)
                eng = nc.sync if jb % 2 == 0 else nc.scalar
                eng.dma_start(gt2[:], g2v[jb])
                g2tiles.append(gt2)
            if True:
                for it in range(NIT):
                    for jb in range(NCORES):
                        for k in range(2):
                            jj = 2 * jb + k
                            lhs = adjd(jj)[:, :, it * 128:(it + 1) * 128]
                            nc.tensor.matmul(
                                ps2[:, it, 0:C264], lhs,
                                g2tiles[jb][:, k, :, :],
                                start=(jb == 0 and k == 0),
                                stop=(jb == NCORES - 1 and k == 1),
                                perf_mode=DR)
                # inline epilogue per i-tile, spread across engines so the
                # serial tail after the last matmul stays short
                r2 = p_f.tile([128, 1], dt.float32, tag="r2", name="r2",
                              bufs=2)
                nc.vector.reciprocal(r2[:], ps2[:, it, NCLS:G2C])
                z = p_f.tile([128, NCLS], dt.float32, tag="z2", name="z2",
                             bufs=2)
                nc.vector.tensor_scalar_mul(z[:], ps2[:, it, 0:NCLS], r2[:])
                e = p_f.tile([128, NCLS], dt.float32, tag="e2", name="e2",
                             bufs=2)
                nc.scalar.activation(e[:], z[:], AF.Exp)
                nc.vector.tensor_scalar(e[:], e[:], 1.0, -1.0, ALU.min,
                                        ALU.add)
                o = p_f.tile([128, NCLS], dt.float32, tag="o2", name="o2",
                             bufs=2)
                nc.vector.scalar_tensor_tensor(o[:], z[:], 0.0, e[:],
                                               ALU.max, ALU.add)
                # log_softmax without max subtraction (o <= ~10)
                t4 = p_f.tile([128, NCLS], dt.float32, tag="t4", name="t4",
                              bufs=2)
                ssum = p_f.tile([128, 1], dt.float32, tag="ssum",
                                name="ssum", bufs=2)
                nc.scalar.activation(t4[:], o[:], AF.Exp, accum_out=ssum[:])
                lg = p_f.tile([128, 1], dt.float32, tag="lg", name="lg",
                              bufs=2)
                nc.scalar.activation(lg[:], ssum[:], AF.Ln)
                fin = p_f.tile([128, NCLS], dt.float32, tag="fin",
                               name="fin", bufs=2)
                nc.vector.tensor_scalar(fin[:], o[:], lg[:], None,
                                        ALU.subtract)
                nc.sync.dma_start(out_d[it * 128:(it + 1) * 128, :], fin[:])

    nc.finalize()
    return nc


_CACHE = {}


def _pair(a):
    hi = a.astype(BF16)
    lo = (a - hi.astype(np.float32)).astype(BF16)
    return hi, lo


def prepare_inputs(x, adj, W_heads, a_heads, W_out, a_out):
    """Shard + lay out the full inputs for the 8 cores."""
    x2 = np.asarray(x, np.float32)[0]          # [N, F]
    adj2 = np.asarray(adj)[0]                  # [N, N] int32
    W3 = np.asarray(W_heads, np.float32).reshape(NH, F, HID)
    a3 = np.asarray(a_heads, np.float32)       # [NH, 2*HID, 1]
    Wo = np.asarray(W_out, np.float32).reshape(GH_TOT, NCLS)
    ao = np.asarray(a_out, np.float32)         # [2*NCLS, 1]

    # fold the edge-score projections into the weights:
    #   s2 = x @ (W @ a2),   s2' = xcat @ (Wo @ ao2)
    u = np.einsum("hfo,ho->hf", W3.astype(np.float64),
                  a3[:, HID:, 0].astype(np.float64)).astype(np.float32)
    u_hi, u_lo = _pair(u)
    U6 = np.zeros((F, 8), BF16)
    for h in range(NH):
        U6[:, 2 * h] = u_hi[h]
        U6[:, 2 * h + 1] = u_lo[h]
    u2 = (Wo.astype(np.float64)
          @ ao[NCLS:, 0].astype(np.float64)).astype(np.float32)
    Wo_ext = np.concatenate([Wo, u2[:, None]], axis=1)       # [GH, 257]
    # [p, ct, c] packed so the Wo DMA is one contiguous run per partition
    Wo_b = np.ascontiguousarray(
        Wo_ext.astype(F8E4).reshape(NCT // 2, 2, 128, G2C)
        .transpose(2, 0, 1, 3)
    ).reshape(128, NCT * G2C)
    # [p, h, ft, o] packed W
    W_b = np.ascontiguousarray(
        W3.astype(BF16).reshape(NH, NFT, 128, HID).transpose(2, 0, 1, 3)
    ).reshape(128, NH * NFT * HID)
    xT = np.ascontiguousarray(x2.T)            # [F, N]
    adj8 = (adj2 > 0).astype(F8E4)             # exact 0/1

    # exact per-head max of s2 = x @ u, folded on the host so the device
    # needs no max-reduction collective.  Mirror the device arithmetic
    # (bf16 x_hi against the u hi/lo pair, accumulated in fp32).
    xh_f = x2.astype(BF16).astype(np.float32)
    s2 = (xh_f @ u_hi.T.astype(np.float32)
          + xh_f @ u_lo.T.astype(np.float32))                     # [N, NH]
    negC = np.broadcast_to(
        -s2.max(axis=0, keepdims=True), (128, NH)
    ).astype(np.float32)                                          # [128, NH]

    in_maps = []
    for c in range(NCORES):
        sl = slice(c * SLAB, (c + 1) * SLAB)
        # [p, ft, i] packed x slab
        xh = np.ascontiguousarray(
            xT[:, sl].astype(BF16).reshape(NFT, 128, SLAB).transpose(1, 0, 2)
        ).reshape(128, NFT * SLAB)
        # DR-interleaved adjacency: [half, p, jj, i, n] with j decomposed as
        # half*2048 + jj*256 + i*128 + p
        adjTc = np.ascontiguousarray(adj8[sl, :].T)          # [N, SLAB]
        adj8i = np.ascontiguousarray(
            adjTc.reshape(2, NJJ // 2, 2, 128, SLAB).transpose(0, 3, 1, 2, 4)
        ).reshape(2 * 128, NJJ // 2 * 2 * SLAB)
        in_maps.append({
            "adj8": adj8i,
            "xT_hi": xh,
            "U6": U6, "negC": negC,
            "W": W_b, "Wo": Wo_b,
            "ident": np.eye(128, dtype=BF16),
        })
    return in_maps


def kernel(x, adj, W_heads, a_heads, W_out, a_out):
    if "nc" not in _CACHE:
        # touch the devices once so any residual bad state from a previous
        # process surfaces (and clears) before the real run
        try:
            import jax
            jax.block_until_ready(jax.numpy.zeros(8))
        except Exception:
            pass
        _CACHE["nc"] = build()
    nc = _CACHE["nc"]
    in_maps = prepare_inputs(x, adj, W_heads, a_heads, W_out, a_out)
    res = run_bass_kernel_spmd(nc, in_maps, list(range(NCORES)))
    out = np.concatenate([res.results[c]["out"] for c in range(NCORES)],
                         axis=0)
    return out.reshape(1, N, NCLS)
